# revision 13
# baseline (speedup 1.0000x reference)
"""GQA attention layer (B=1, S=2048, D=4096, H=32, KV=8, HD=128) on 8 TRN2
NeuronCores. Tensor-parallel over the 8 KV-head groups; per-head AllGather of
the attention outputs (overlapped with attention compute), then each core
computes one column shard of the output projection (no AllReduce needed).

All matmuls run in float32r (full-rate fp32 mode, ~1.5e-4 rel err).
Attention is computed in transposed orientation (scores^T = kT-slices as lhsT
against qT) so softmax sums become TensorEngine matmuls and no P-transposes
are needed. RoPE pairs are de-interleaved by permuting wq/wk columns
host-side; all DRAM operands are host-pretiled for contiguous DMA.
"""
import numpy as np
import ml_dtypes

S = 2048
D = 4096
HD = 128
QH = 4            # q heads per core
NCORES = 8
ST = S // 128     # 16 s-tiles
DK = D // 128     # 32 contraction tiles
G = 4             # q groups per head
SG = S // G       # 512 q positions per group
SCALE = 1.0 / np.sqrt(128.0)

_CACHE = {}


def _build():
    import concourse.mybir as mybir
    import concourse.tile as tile
    from concourse import bacc

    f32, f32r = mybir.dt.float32, mybir.dt.float32r
    bf16 = mybir.dt.bfloat16
    nc = bacc.Bacc("TRN2", target_bir_lowering=False, debug=False,
                   num_devices=NCORES)

    xt = nc.dram_tensor("xt", [ST, 128, DK, 128], f32r, kind="ExternalInput").ap()
    wq = nc.dram_tensor("wq", [128, DK, QH * HD], f32r, kind="ExternalInput").ap()
    wkv = nc.dram_tensor("wkv", [128, DK, 2 * HD], f32r, kind="ExternalInput").ap()
    wo = nc.dram_tensor("wo", [128, DK, 512], bf16, kind="ExternalInput").ap()
    cos = nc.dram_tensor("cos", [128, ST, 64], f32, kind="ExternalInput").ap()
    sin = nc.dram_tensor("sin", [128, ST, 64], f32, kind="ExternalInput").ap()
    tri = nc.dram_tensor("tri", [128, 4, SG], f32, kind="ExternalInput").ap()
    onesc = nc.dram_tensor("onesc", [128, 1], f32r, kind="ExternalInput").ap()
    onesr = nc.dram_tensor("onesr", [1, 128], f32r, kind="ExternalInput").ap()
    ident = nc.dram_tensor("ident", [128, 128], f32r, kind="ExternalInput").ap()
    yt = nc.dram_tensor("yt", [512, S], f32, kind="ExternalOutput").ap()

    with tile.TileContext(nc) as tc:
        with (
            tc.tile_pool(name="const", bufs=1) as constp,
            tc.tile_pool(name="resid", bufs=1) as resid,
            tc.tile_pool(name="dram", bufs=1, space="DRAM") as dram,
            tc.tile_pool(name="dram_l", bufs=4, space="DRAM") as dram_l,
        ):
            cos_sb = constp.tile([128, ST, 64], f32)
            sin_sb = constp.tile([128, ST, 64], f32)
            tri_sb = constp.tile([128, 4, SG], f32)
            onesc_sb = constp.tile([128, 1], f32r)
            onesr_sb = constp.tile([1, 128], f32r)
            ident_sb = constp.tile([128, 128], f32r)
            nc.sync.dma_start(cos_sb[:], cos)
            nc.sync.dma_start(sin_sb[:], sin)
            nc.sync.dma_start(tri_sb[:], tri)
            nc.sync.dma_start(onesc_sb[:], onesc)
            nc.sync.dma_start(onesr_sb[:], onesr)
            nc.sync.dma_start(ident_sb[:], ident)

            # residents across phases
            kt_sb = resid.tile([128, S], f32r)           # roped K^T  [d, s]
            v_sb = resid.tile([128, ST, HD], f32r)       # natural V  [s-in-tile, st, d]

            qt_spill = dram.tile([QH * 128, S], f32r)    # roped Q^T per head
            cc_in = [dram.tile([128, S], bf16, name=f"cc_in{h}") for h in range(QH)]
            cc_out = [dram.tile([NCORES * 128, S], bf16, addr_space="Shared",
                                name=f"cc_out{h}") for h in range(QH)]

            # ---------------- Phase 1: QKV projections + RoPE + transposes
            with (
                tc.tile_pool(name="wpool", bufs=1) as wpool,
                tc.tile_pool(name="xpool", bufs=2) as xpool,
                tc.tile_pool(name="p1tmp", bufs=3) as p1tmp,
                tc.tile_pool(name="p1psum", bufs=2, space="PSUM") as p1psum,
            ):
                wq_sb = wpool.tile([128, DK, QH * HD], f32r)
                wkv_sb = wpool.tile([128, DK, 2 * HD], f32r)
                for kt in range(DK):
                    nc.sync.dma_start(wq_sb[:, kt, :], wq[:, kt, :])
                    nc.sync.dma_start(wkv_sb[:, kt, :], wkv[:, kt, :])

                def emit_transposes(qnat, knat, ssl):
                    # deferred one s-tile so PE never waits on the DVE rope
                    qts = p1tmp.tile([128, QH, 128], f32r, name="qts")
                    for h in range(QH):
                        qtr = p1psum.tile([128, 128], f32r, name="qtr")
                        nc.tensor.transpose(qtr[:], qnat[:, h, :], ident_sb[:])
                        nc.vector.tensor_copy(qts[:, h, :], qtr[:])
                    nc.sync.dma_start(
                        qt_spill.rearrange("(h p) s -> p h s", p=128)[:, :, ssl],
                        qts[:])
                    ktr = p1psum.tile([128, 128], f32r, name="ktr")
                    nc.tensor.transpose(ktr[:], knat[:], ident_sb[:])
                    nc.vector.tensor_copy(kt_sb[:, ssl], ktr[:])

                pending = None
                for st in range(ST):
                    ssl = slice(st * 128, (st + 1) * 128)
                    xt_st = xpool.tile([128, DK, 128], f32r, name="xt_st")
                    nc.sync.dma_start(xt_st[:], xt[st])

                    q_ps = p1psum.tile([128, QH * HD], f32, name="q_ps")
                    kv_ps = p1psum.tile([128, 2 * HD], f32, name="kv_ps")
                    for kt in range(DK):
                        nc.tensor.matmul(q_ps[:], lhsT=xt_st[:, kt], rhs=wq_sb[:, kt],
                                         start=(kt == 0), stop=(kt == DK - 1))
                    for kt in range(DK):
                        nc.tensor.matmul(kv_ps[:], lhsT=xt_st[:, kt], rhs=wkv_sb[:, kt],
                                         start=(kt == 0), stop=(kt == DK - 1))
                    if pending is not None:
                        emit_transposes(*pending)

                    # RoPE on q (4 heads batched) during PSUM eviction.
                    qp = q_ps[:].rearrange("p (h d) -> p h d", h=QH)
                    qa, qb = qp[:, :, 0:64], qp[:, :, 64:128]
                    cbc = cos_sb[:, st:st + 1, :].to_broadcast([128, QH, 64])
                    sbc = sin_sb[:, st:st + 1, :].to_broadcast([128, QH, 64])
                    t1 = p1tmp.tile([128, QH, 64], f32, name="t1")
                    t2 = p1tmp.tile([128, QH, 64], f32, name="t2")
                    qnat = p1tmp.tile([128, QH, HD], f32r, name="qnat")
                    na, nb = qnat[:, :, 0:64], qnat[:, :, 64:128]
                    nc.vector.tensor_tensor(t1[:], qa, sbc, mybir.AluOpType.mult)
                    nc.vector.tensor_tensor(t2[:], qb, sbc, mybir.AluOpType.mult)
                    nc.vector.tensor_tensor(na, qa, cbc, mybir.AluOpType.mult)
                    nc.vector.tensor_tensor(nb, qb, cbc, mybir.AluOpType.mult)
                    nc.vector.tensor_tensor(na, na, t2[:], mybir.AluOpType.subtract)
                    nc.vector.tensor_tensor(nb, nb, t1[:], mybir.AluOpType.add)

                    # RoPE on k
                    ka, kb = kv_ps[:, 0:64], kv_ps[:, 64:128]
                    cb1 = cos_sb[:, st, :]
                    sb1 = sin_sb[:, st, :]
                    kt1 = p1tmp.tile([128, 64], f32, name="kt1")
                    kt2 = p1tmp.tile([128, 64], f32, name="kt2")
                    knat = p1tmp.tile([128, HD], f32r, name="knat")
                    kna, knb = knat[:, 0:64], knat[:, 64:128]
                    nc.vector.tensor_tensor(kt1[:], ka, sb1, mybir.AluOpType.mult)
                    nc.vector.tensor_tensor(kt2[:], kb, sb1, mybir.AluOpType.mult)
                    nc.vector.tensor_tensor(kna, ka, cb1, mybir.AluOpType.mult)
                    nc.vector.tensor_tensor(knb, kb, cb1, mybir.AluOpType.mult)
                    nc.vector.tensor_tensor(kna, kna, kt2[:], mybir.AluOpType.subtract)
                    nc.vector.tensor_tensor(knb, knb, kt1[:], mybir.AluOpType.add)

                    # V natural, straight copy
                    nc.vector.tensor_copy(v_sb[:, st, :], kv_ps[:, HD:2 * HD])

                    pending = (qnat, knat, ssl)
                emit_transposes(*pending)

            # ---------------- Phase 2: attention per (head, group) + wo preload
            with tc.tile_pool(name="wopool", bufs=1) as wopool:
                wo_sb = wopool.tile([128, DK, 512], bf16)
                nc.sync.dma_start(wo_sb[:], wo)

                with (
                    tc.tile_pool(name="p2tmp", bufs=3) as p2tmp,
                    tc.tile_pool(name="p2lb", bufs=2) as p2lb,
                    tc.tile_pool(name="p2psum", bufs=2, space="PSUM") as p2psum,
                    tc.tile_pool(name="p2opsum", bufs=2, space="PSUM") as p2opsum,
                ):
                    for h in range(QH):
                        for g in range(G):
                            gsl = slice(g * SG, (g + 1) * SG)
                            qt_g = p2tmp.tile([128, SG], f32r, name="qt_g")
                            nc.sync.dma_start(
                                qt_g[:], qt_spill[h * 128:(h + 1) * 128, gsl])

                            ot_ps = p2opsum.tile([128, SG], f32, name="ot_ps", bufs=3)
                            l_ps = p2opsum.tile([1, SG], f32, name="l_ps")
                            nk = 4 * g + 4
                            DEPTH = 3
                            st_tiles = {}

                            def do_st(j, qt_g=qt_g, st_tiles=st_tiles):
                                stp = p2psum.tile([128, SG], f32, name="st_ps",
                                                  tag="st_ps", bufs=DEPTH)
                                nc.tensor.matmul(
                                    stp[:], lhsT=kt_sb[:, j * 128:(j + 1) * 128],
                                    rhs=qt_g[:], start=True, stop=True)
                                st_tiles[j] = stp

                            for j in range(min(DEPTH, nk)):
                                do_st(j)
                            for j in range(nk):
                                st_ps = st_tiles.pop(j)
                                put = p2tmp.tile([128, SG], f32r, name="put")
                                nc.scalar.activation(put[:], st_ps[:],
                                                     mybir.ActivationFunctionType.Exp,
                                                     scale=SCALE)
                                if j >= 4 * g:
                                    nc.vector.tensor_tensor(put[:], put[:],
                                                            tri_sb[:, j - 4 * g, :],
                                                            mybir.AluOpType.mult)
                                nc.tensor.matmul(ot_ps[:], lhsT=v_sb[:, j, :],
                                                 rhs=put[:],
                                                 start=(j == 0), stop=(j == nk - 1))
                                nc.tensor.matmul(l_ps[:], lhsT=onesc_sb[:],
                                                 rhs=put[:],
                                                 start=(j == 0), stop=(j == nk - 1))
                                if j + DEPTH < nk:
                                    do_st(j + DEPTH)

                            linv_f = p2lb.tile([1, SG], f32, name="linv_f")
                            nc.vector.reciprocal_approx_fast(linv_f[:], l_ps[:])
                            linv_r = p2lb.tile([1, SG], f32r, name="linv_r")
                            nc.vector.tensor_copy(linv_r[:], linv_f[:])
                            lb_ps = p2psum.tile([128, SG], f32, name="st_ps",
                                                tag="st_ps", bufs=DEPTH)
                            nc.tensor.matmul(lb_ps[:], lhsT=onesr_sb[:],
                                             rhs=linv_r[:], start=True, stop=True)
                            lb_sb = p2lb.tile([128, SG], f32, name="lb_sb")
                            nc.vector.tensor_copy(lb_sb[:], lb_ps[:])
                            on_sb = p2tmp.tile([128, SG], bf16, name="on_sb")
                            nc.vector.tensor_tensor(on_sb[:], ot_ps[:], lb_sb[:],
                                                    mybir.AluOpType.mult)
                            nc.sync.dma_start(cc_in[h][:, gsl], on_sb[:])

                        nc.gpsimd.collective_compute(
                            "AllGather", mybir.AluOpType.bypass,
                            ins=[cc_in[h].opt()], outs=[cc_out[h].opt()],
                            replica_groups=[list(range(NCORES))],
                        )

                # ---------------- Phase 3: yT = wo^T-contract @ O^T_full
                with (
                    tc.tile_pool(name="p3tmp", bufs=3) as p3tmp,
                    tc.tile_pool(name="p3out", bufs=2) as p3out,
                    tc.tile_pool(name="p3psum", bufs=2, space="PSUM") as p3psum,
                ):
                    for sq in range(4):
                        sqsl = slice(sq * 512, (sq + 1) * 512)
                        y_ps = p3psum.tile([128, 4, 512], f32, name="y_ps")
                        for h in range(QH):
                            ot_h = p3tmp.tile([128, NCORES, 512], bf16, name="ot_h")
                            nc.sync.dma_start(
                                ot_h[:],
                                cc_out[h].rearrange("(r p) s -> p r s",
                                                    p=128)[:, :, sqsl])
                            for r in range(NCORES):
                                kt2 = h * NCORES + r
                                for dt in range(4):
                                    nc.tensor.matmul(
                                        y_ps[:, dt, :],
                                        lhsT=wo_sb[:, kt2, dt * 128:(dt + 1) * 128],
                                        rhs=ot_h[:, r, :],
                                        start=(kt2 == 0), stop=(kt2 == DK - 1))
                        for dt in range(4):
                            y_sb = p3out.tile([128, 512], f32, name="y_sb")
                            nc.vector.tensor_copy(y_sb[:], y_ps[:, dt, :])
                            nc.sync.dma_start(yt[dt * 128:(dt + 1) * 128, sqsl],
                                              y_sb[:])
    nc.compile()
    return nc


def _host_prep(inputs):
    x = np.asarray(inputs["x"], dtype=np.float32)
    wq = np.asarray(inputs["wq"], dtype=np.float32)
    wk = np.asarray(inputs["wk"], dtype=np.float32)
    wv = np.asarray(inputs["wv"], dtype=np.float32)
    wo = np.asarray(inputs["wo"], dtype=np.float32)
    cos = np.asarray(inputs["freqs_cos"], dtype=np.float32)
    sin = np.asarray(inputs["freqs_sin"], dtype=np.float32)
    mask = np.asarray(inputs["mask"], dtype=np.float32)

    # xt[st, p, kt, s] = x[128*st + s, 128*kt + p]
    xts = np.ascontiguousarray(
        x.reshape(ST, 128, DK, 128).transpose(0, 3, 2, 1))

    # de-interleave RoPE pairs within each head: evens then odds
    perm = np.concatenate([np.arange(0, HD, 2), np.arange(1, HD, 2)])

    cos_t = np.ascontiguousarray(cos.reshape(ST, 128, 64).transpose(1, 0, 2))
    sin_t = np.ascontiguousarray(sin.reshape(ST, 128, 64).transpose(1, 0, 2))

    # causal tile masks from the actual mask input (g-independent for causal)
    trif = np.empty((4, 128, SG), dtype=np.float32)
    for r in range(4):
        trif[r] = (mask[0:SG, 128 * r:128 * (r + 1)].T == 0.0).astype(np.float32)
    tri_t = np.ascontiguousarray(trif.transpose(1, 0, 2))

    def ktile(w):  # [D, m] -> [128, DK, m]
        return np.ascontiguousarray(
            w.reshape(DK, 128, w.shape[1]).transpose(1, 0, 2))

    in_maps = []
    for c in range(NCORES):
        wq_c = wq[:, 512 * c:512 * (c + 1)].reshape(D, QH, HD)[:, :, perm]
        wq_c = wq_c.reshape(D, QH * HD)
        wk_c = wk[:, 128 * c:128 * (c + 1)][:, perm]
        wv_c = wv[:, 128 * c:128 * (c + 1)]
        wkv_c = np.concatenate([wk_c, wv_c], axis=1)
        # wo rows reordered to (head, rank, d) to match per-head AllGather
        wo_c = wo[:, 512 * c:512 * (c + 1)]
        wo_c = wo_c.reshape(NCORES, QH, 128, 512).transpose(1, 0, 2, 3)
        wo_c = wo_c.reshape(D, 512)
        in_maps.append({
            "xt": xts,
            "wq": ktile(wq_c),
            "wkv": ktile(wkv_c),
            "wo": ktile(wo_c).astype(ml_dtypes.bfloat16),
            "cos": cos_t,
            "sin": sin_t,
            "tri": tri_t,
            "onesc": np.ones((128, 1), dtype=np.float32),
            "onesr": np.ones((1, 128), dtype=np.float32),
            "ident": np.eye(128, dtype=np.float32),
        })
    return in_maps


def _run(inputs, trace=False, tmpdir=None):
    from concourse import bass_utils
    if "nc" not in _CACHE:
        _CACHE["nc"] = _build()
    nc = _CACHE["nc"]
    in_maps = _host_prep(inputs)
    res = bass_utils.run_bass_kernel_spmd(
        nc, in_maps, core_ids=list(range(NCORES)), trace=trace, tmpdir=tmpdir)
    yts = [res.results[c]["yt"] for c in range(NCORES)]
    y = np.concatenate([t.T for t in yts], axis=1).astype(np.float32)
    return y.reshape(1, S, D), res


def kernel(**inputs):
    y, _ = _run(inputs, trace=False)
    return y


# revision 14
# speedup vs baseline: 1.1945x; 1.1945x over previous
"""GQA attention layer (B=1, S=2048, D=4096, H=32, KV=8, HD=128) on 8 TRN2
NeuronCores. Tensor-parallel over the 8 KV-head groups; per-head AllGather of
the attention outputs (overlapped with attention compute), then each core
computes one column shard of the output projection (no AllReduce needed).

All matmuls run in float32r (full-rate fp32 mode, ~1.5e-4 rel err).
Attention is computed in transposed orientation (scores^T = kT-slices as lhsT
against qT) so softmax sums become TensorEngine matmuls and no P-transposes
are needed. RoPE pairs are de-interleaved by permuting wq/wk columns
host-side; all DRAM operands are host-pretiled for contiguous DMA.
"""
import numpy as np
import ml_dtypes

S = 2048
D = 4096
HD = 128
QH = 4            # q heads per core
NCORES = 8
ST = S // 128     # 16 s-tiles
DK = D // 128     # 32 contraction tiles
G = 4             # q groups per head
SG = S // G       # 512 q positions per group
SCALE = 1.0 / np.sqrt(128.0)

_CACHE = {}


def _build():
    import concourse.mybir as mybir
    import concourse.tile as tile
    from concourse import bacc

    f32, f32r = mybir.dt.float32, mybir.dt.float32r
    bf16 = mybir.dt.bfloat16
    nc = bacc.Bacc("TRN2", target_bir_lowering=False, debug=False,
                   num_devices=NCORES)

    xt = nc.dram_tensor("xt", [ST, 128, DK, 128], f32r, kind="ExternalInput").ap()
    wq = nc.dram_tensor("wq", [128, DK, QH * HD], f32r, kind="ExternalInput").ap()
    wkv = nc.dram_tensor("wkv", [128, DK, 2 * HD], f32r, kind="ExternalInput").ap()
    wo = nc.dram_tensor("wo", [128, DK, 512], bf16, kind="ExternalInput").ap()
    cos = nc.dram_tensor("cos", [128, ST, 64], f32, kind="ExternalInput").ap()
    sin = nc.dram_tensor("sin", [128, ST, 64], f32, kind="ExternalInput").ap()
    tri = nc.dram_tensor("tri", [128, 4, SG], f32, kind="ExternalInput").ap()
    onesc = nc.dram_tensor("onesc", [128, 1], f32r, kind="ExternalInput").ap()
    onesr = nc.dram_tensor("onesr", [1, 128], f32r, kind="ExternalInput").ap()
    ident = nc.dram_tensor("ident", [128, 128], f32r, kind="ExternalInput").ap()
    yt = nc.dram_tensor("yt", [512, S], f32, kind="ExternalOutput").ap()

    with tile.TileContext(nc) as tc:
        with (
            tc.tile_pool(name="const", bufs=1) as constp,
            tc.tile_pool(name="resid", bufs=1) as resid,
            tc.tile_pool(name="dram", bufs=1, space="DRAM") as dram,
            tc.tile_pool(name="dram_l", bufs=4, space="DRAM") as dram_l,
        ):
            cos_sb = constp.tile([128, ST, 64], f32)
            sin_sb = constp.tile([128, ST, 64], f32)
            tri_sb = constp.tile([128, 4, SG], f32)
            onesc_sb = constp.tile([128, 1], f32r)
            onesr_sb = constp.tile([1, 128], f32r)
            ident_sb = constp.tile([128, 128], f32r)
            nc.sync.dma_start(cos_sb[:], cos)
            nc.sync.dma_start(sin_sb[:], sin)
            nc.sync.dma_start(tri_sb[:], tri)
            nc.sync.dma_start(onesc_sb[:], onesc)
            nc.sync.dma_start(onesr_sb[:], onesr)
            nc.sync.dma_start(ident_sb[:], ident)

            # residents across phases
            kt_sb = resid.tile([128, S], f32r)           # roped K^T  [d, s]
            v_sb = resid.tile([128, ST, HD], f32r)       # natural V  [s-in-tile, st, d]

            qt_spill = dram.tile([QH * 128, S], f32r)    # roped Q^T per head
            cc_in = [dram.tile([128, S], bf16, name=f"cc_in{h}") for h in range(QH)]
            cc_out = [dram.tile([NCORES * 128, S], bf16, addr_space="Shared",
                                name=f"cc_out{h}") for h in range(QH)]

            # ---------------- Phase 1: QKV projections + RoPE + transposes
            with (
                tc.tile_pool(name="wpool", bufs=1) as wpool,
                tc.tile_pool(name="xpool", bufs=2) as xpool,
                tc.tile_pool(name="p1tmp", bufs=3) as p1tmp,
                tc.tile_pool(name="p1psum", bufs=2, space="PSUM") as p1psum,
            ):
                wq_sb = wpool.tile([128, DK, QH * HD], f32r)
                wkv_sb = wpool.tile([128, DK, 2 * HD], f32r)
                for kt in range(DK):
                    nc.sync.dma_start(wq_sb[:, kt, :], wq[:, kt, :])
                    nc.sync.dma_start(wkv_sb[:, kt, :], wkv[:, kt, :])

                def emit_transposes(qnat, knat, ssl):
                    # deferred one s-tile so PE never waits on the DVE rope
                    qts = p1tmp.tile([128, QH, 128], f32r, name="qts")
                    for h in range(QH):
                        qtr = p1psum.tile([128, 128], f32r, name="qtr")
                        nc.tensor.transpose(qtr[:], qnat[:, h, :], ident_sb[:])
                        nc.vector.tensor_copy(qts[:, h, :], qtr[:])
                    nc.sync.dma_start(
                        qt_spill.rearrange("(h p) s -> p h s", p=128)[:, :, ssl],
                        qts[:])
                    ktr = p1psum.tile([128, 128], f32r, name="ktr")
                    nc.tensor.transpose(ktr[:], knat[:], ident_sb[:])
                    nc.vector.tensor_copy(kt_sb[:, ssl], ktr[:])

                pending = None
                for st in range(ST):
                    ssl = slice(st * 128, (st + 1) * 128)
                    xt_st = xpool.tile([128, DK, 128], f32r, name="xt_st")
                    nc.sync.dma_start(xt_st[:], xt[st])

                    q_ps = p1psum.tile([128, QH * HD], f32, name="q_ps")
                    kv_ps = p1psum.tile([128, 2 * HD], f32, name="kv_ps")
                    for kt in range(DK):
                        nc.tensor.matmul(q_ps[:], lhsT=xt_st[:, kt], rhs=wq_sb[:, kt],
                                         start=(kt == 0), stop=(kt == DK - 1))
                    for kt in range(DK):
                        nc.tensor.matmul(kv_ps[:], lhsT=xt_st[:, kt], rhs=wkv_sb[:, kt],
                                         start=(kt == 0), stop=(kt == DK - 1))
                    if pending is not None:
                        emit_transposes(*pending)

                    # RoPE on q (4 heads batched) during PSUM eviction.
                    qp = q_ps[:].rearrange("p (h d) -> p h d", h=QH)
                    qa, qb = qp[:, :, 0:64], qp[:, :, 64:128]
                    cbc = cos_sb[:, st:st + 1, :].to_broadcast([128, QH, 64])
                    sbc = sin_sb[:, st:st + 1, :].to_broadcast([128, QH, 64])
                    t1 = p1tmp.tile([128, QH, 64], f32, name="t1")
                    t2 = p1tmp.tile([128, QH, 64], f32, name="t2")
                    qnat = p1tmp.tile([128, QH, HD], f32r, name="qnat")
                    na, nb = qnat[:, :, 0:64], qnat[:, :, 64:128]
                    nc.vector.tensor_tensor(t1[:], qa, sbc, mybir.AluOpType.mult)
                    nc.vector.tensor_tensor(t2[:], qb, sbc, mybir.AluOpType.mult)
                    nc.vector.tensor_tensor(na, qa, cbc, mybir.AluOpType.mult)
                    nc.vector.tensor_tensor(nb, qb, cbc, mybir.AluOpType.mult)
                    nc.vector.tensor_tensor(na, na, t2[:], mybir.AluOpType.subtract)
                    nc.vector.tensor_tensor(nb, nb, t1[:], mybir.AluOpType.add)

                    # RoPE on k
                    ka, kb = kv_ps[:, 0:64], kv_ps[:, 64:128]
                    cb1 = cos_sb[:, st, :]
                    sb1 = sin_sb[:, st, :]
                    kt1 = p1tmp.tile([128, 64], f32, name="kt1")
                    kt2 = p1tmp.tile([128, 64], f32, name="kt2")
                    knat = p1tmp.tile([128, HD], f32r, name="knat")
                    kna, knb = knat[:, 0:64], knat[:, 64:128]
                    nc.vector.tensor_tensor(kt1[:], ka, sb1, mybir.AluOpType.mult)
                    nc.vector.tensor_tensor(kt2[:], kb, sb1, mybir.AluOpType.mult)
                    nc.vector.tensor_tensor(kna, ka, cb1, mybir.AluOpType.mult)
                    nc.vector.tensor_tensor(knb, kb, cb1, mybir.AluOpType.mult)
                    nc.vector.tensor_tensor(kna, kna, kt2[:], mybir.AluOpType.subtract)
                    nc.vector.tensor_tensor(knb, knb, kt1[:], mybir.AluOpType.add)

                    # V natural, straight copy
                    nc.vector.tensor_copy(v_sb[:, st, :], kv_ps[:, HD:2 * HD])

                    pending = (qnat, knat, ssl)
                emit_transposes(*pending)

            # ---------------- Phase 2: attention per (head, group) + wo preload
            with tc.tile_pool(name="wopool", bufs=1) as wopool:
                wo_sb = wopool.tile([128, DK, 512], bf16)
                nc.sync.dma_start(wo_sb[:], wo)

                with (
                    tc.tile_pool(name="p2tmp", bufs=3) as p2tmp,
                    tc.tile_pool(name="p2lb", bufs=2) as p2lb,
                    tc.tile_pool(name="p2psum", bufs=2, space="PSUM") as p2psum,
                    tc.tile_pool(name="p2opsum", bufs=2, space="PSUM") as p2opsum,
                ):
                    for h in range(QH):
                        for g in range(G):
                            gsl = slice(g * SG, (g + 1) * SG)
                            qt_g = p2tmp.tile([128, SG], f32r, name="qt_g")
                            nc.sync.dma_start(
                                qt_g[:], qt_spill[h * 128:(h + 1) * 128, gsl])

                            ot_ps = p2opsum.tile([128, SG], f32, name="ot_ps", bufs=2)
                            l_ps = p2opsum.tile([1, SG], f32, name="l_ps")
                            nk = 4 * g + 4
                            DEPTH = 3
                            st_tiles = {}

                            def do_st(j, qt_g=qt_g, st_tiles=st_tiles):
                                stp = p2psum.tile([128, SG], f32, name="st_ps",
                                                  tag="st_ps", bufs=DEPTH)
                                nc.tensor.matmul(
                                    stp[:], lhsT=kt_sb[:, j * 128:(j + 1) * 128],
                                    rhs=qt_g[:], start=True, stop=True)
                                st_tiles[j] = stp

                            for j in range(min(DEPTH, nk)):
                                do_st(j)
                            for j in range(nk):
                                st_ps = st_tiles.pop(j)
                                put = p2tmp.tile([128, SG], f32r, name="put")
                                nc.scalar.activation(put[:], st_ps[:],
                                                     mybir.ActivationFunctionType.Exp,
                                                     scale=SCALE)
                                if j >= 4 * g:
                                    nc.vector.tensor_tensor(put[:], put[:],
                                                            tri_sb[:, j - 4 * g, :],
                                                            mybir.AluOpType.mult)
                                nc.tensor.matmul(ot_ps[:], lhsT=v_sb[:, j, :],
                                                 rhs=put[:],
                                                 start=(j == 0), stop=(j == nk - 1))
                                nc.tensor.matmul(l_ps[:], lhsT=onesc_sb[:],
                                                 rhs=put[:],
                                                 start=(j == 0), stop=(j == nk - 1))
                                if j + DEPTH < nk:
                                    do_st(j + DEPTH)

                            linv_f = p2lb.tile([1, SG], f32, name="linv_f")
                            nc.vector.reciprocal_approx_fast(linv_f[:], l_ps[:])
                            linv_r = p2lb.tile([1, SG], f32r, name="linv_r")
                            nc.vector.tensor_copy(linv_r[:], linv_f[:])
                            lb_ps = p2psum.tile([128, SG], f32, name="lb_ps",
                                                tag="lb_ps", bufs=1)
                            nc.tensor.matmul(lb_ps[:], lhsT=onesr_sb[:],
                                             rhs=linv_r[:], start=True, stop=True)
                            lb_sb = p2lb.tile([128, SG], f32, name="lb_sb")
                            nc.vector.tensor_copy(lb_sb[:], lb_ps[:])
                            on_sb = p2tmp.tile([128, SG], bf16, name="on_sb")
                            nc.vector.tensor_tensor(on_sb[:], ot_ps[:], lb_sb[:],
                                                    mybir.AluOpType.mult)
                            nc.sync.dma_start(cc_in[h][:, gsl], on_sb[:])

                        nc.gpsimd.collective_compute(
                            "AllGather", mybir.AluOpType.bypass,
                            ins=[cc_in[h].opt()], outs=[cc_out[h].opt()],
                            replica_groups=[list(range(NCORES))],
                        )

                # ---------------- Phase 3: yT = wo^T-contract @ O^T_full
                with (
                    tc.tile_pool(name="p3tmp", bufs=3) as p3tmp,
                    tc.tile_pool(name="p3out", bufs=2) as p3out,
                    tc.tile_pool(name="p3psum", bufs=2, space="PSUM") as p3psum,
                ):
                    for sq in range(4):
                        sqsl = slice(sq * 512, (sq + 1) * 512)
                        y_ps = p3psum.tile([128, 4, 512], f32, name="y_ps")
                        for h in range(QH):
                            ot_h = p3tmp.tile([128, NCORES, 512], bf16, name="ot_h")
                            nc.sync.dma_start(
                                ot_h[:],
                                cc_out[h].rearrange("(r p) s -> p r s",
                                                    p=128)[:, :, sqsl])
                            for r in range(NCORES):
                                kt2 = h * NCORES + r
                                for dt in range(4):
                                    nc.tensor.matmul(
                                        y_ps[:, dt, :],
                                        lhsT=wo_sb[:, kt2, dt * 128:(dt + 1) * 128],
                                        rhs=ot_h[:, r, :],
                                        start=(kt2 == 0), stop=(kt2 == DK - 1))
                        for dt in range(4):
                            y_sb = p3out.tile([128, 512], f32, name="y_sb")
                            nc.vector.tensor_copy(y_sb[:], y_ps[:, dt, :])
                            nc.sync.dma_start(yt[dt * 128:(dt + 1) * 128, sqsl],
                                              y_sb[:])
    nc.compile()
    return nc


def _host_prep(inputs):
    x = np.asarray(inputs["x"], dtype=np.float32)
    wq = np.asarray(inputs["wq"], dtype=np.float32)
    wk = np.asarray(inputs["wk"], dtype=np.float32)
    wv = np.asarray(inputs["wv"], dtype=np.float32)
    wo = np.asarray(inputs["wo"], dtype=np.float32)
    cos = np.asarray(inputs["freqs_cos"], dtype=np.float32)
    sin = np.asarray(inputs["freqs_sin"], dtype=np.float32)
    mask = np.asarray(inputs["mask"], dtype=np.float32)

    # xt[st, p, kt, s] = x[128*st + s, 128*kt + p]
    xts = np.ascontiguousarray(
        x.reshape(ST, 128, DK, 128).transpose(0, 3, 2, 1))

    # de-interleave RoPE pairs within each head: evens then odds
    perm = np.concatenate([np.arange(0, HD, 2), np.arange(1, HD, 2)])

    cos_t = np.ascontiguousarray(cos.reshape(ST, 128, 64).transpose(1, 0, 2))
    sin_t = np.ascontiguousarray(sin.reshape(ST, 128, 64).transpose(1, 0, 2))

    # causal tile masks from the actual mask input (g-independent for causal)
    trif = np.empty((4, 128, SG), dtype=np.float32)
    for r in range(4):
        trif[r] = (mask[0:SG, 128 * r:128 * (r + 1)].T == 0.0).astype(np.float32)
    tri_t = np.ascontiguousarray(trif.transpose(1, 0, 2))

    def ktile(w):  # [D, m] -> [128, DK, m]
        return np.ascontiguousarray(
            w.reshape(DK, 128, w.shape[1]).transpose(1, 0, 2))

    in_maps = []
    for c in range(NCORES):
        wq_c = wq[:, 512 * c:512 * (c + 1)].reshape(D, QH, HD)[:, :, perm]
        wq_c = wq_c.reshape(D, QH * HD)
        wk_c = wk[:, 128 * c:128 * (c + 1)][:, perm]
        wv_c = wv[:, 128 * c:128 * (c + 1)]
        wkv_c = np.concatenate([wk_c, wv_c], axis=1)
        # wo rows reordered to (head, rank, d) to match per-head AllGather
        wo_c = wo[:, 512 * c:512 * (c + 1)]
        wo_c = wo_c.reshape(NCORES, QH, 128, 512).transpose(1, 0, 2, 3)
        wo_c = wo_c.reshape(D, 512)
        in_maps.append({
            "xt": xts,
            "wq": ktile(wq_c),
            "wkv": ktile(wkv_c),
            "wo": ktile(wo_c).astype(ml_dtypes.bfloat16),
            "cos": cos_t,
            "sin": sin_t,
            "tri": tri_t,
            "onesc": np.ones((128, 1), dtype=np.float32),
            "onesr": np.ones((1, 128), dtype=np.float32),
            "ident": np.eye(128, dtype=np.float32),
        })
    return in_maps


def _run(inputs, trace=False, tmpdir=None):
    from concourse import bass_utils
    if "nc" not in _CACHE:
        _CACHE["nc"] = _build()
    nc = _CACHE["nc"]
    in_maps = _host_prep(inputs)
    res = bass_utils.run_bass_kernel_spmd(
        nc, in_maps, core_ids=list(range(NCORES)), trace=trace, tmpdir=tmpdir)
    yts = [res.results[c]["yt"] for c in range(NCORES)]
    y = np.concatenate([t.T for t in yts], axis=1).astype(np.float32)
    return y.reshape(1, S, D), res


def kernel(**inputs):
    y, _ = _run(inputs, trace=False)
    return y


# revision 15
# speedup vs baseline: 1.2140x; 1.0163x over previous
"""GQA attention layer (B=1, S=2048, D=4096, H=32, KV=8, HD=128) on 8 TRN2
NeuronCores. Tensor-parallel over the 8 KV-head groups; per-head AllGather of
the attention outputs (overlapped with attention compute), then each core
computes one column shard of the output projection (no AllReduce needed).

All matmuls run in float32r (full-rate fp32 mode, ~1.5e-4 rel err).
Attention is computed in transposed orientation (scores^T = kT-slices as lhsT
against qT) so softmax sums become TensorEngine matmuls and no P-transposes
are needed. RoPE pairs are de-interleaved by permuting wq/wk columns
host-side; all DRAM operands are host-pretiled for contiguous DMA.
"""
import numpy as np
import ml_dtypes

S = 2048
D = 4096
HD = 128
QH = 4            # q heads per core
NCORES = 8
ST = S // 128     # 16 s-tiles
DK = D // 128     # 32 contraction tiles
G = 4             # q groups per head
SG = S // G       # 512 q positions per group
SCALE = 1.0 / np.sqrt(128.0)

_CACHE = {}


def _build():
    import concourse.mybir as mybir
    import concourse.tile as tile
    from concourse import bacc

    f32, f32r = mybir.dt.float32, mybir.dt.float32r
    bf16 = mybir.dt.bfloat16
    nc = bacc.Bacc("TRN2", target_bir_lowering=False, debug=False,
                   num_devices=NCORES)

    xt = nc.dram_tensor("xt", [ST, 128, DK, 128], bf16, kind="ExternalInput").ap()
    wq = nc.dram_tensor("wq", [128, DK, QH * HD], bf16, kind="ExternalInput").ap()
    wkv = nc.dram_tensor("wkv", [128, DK, 2 * HD], bf16, kind="ExternalInput").ap()
    wo = nc.dram_tensor("wo", [128, DK, 512], bf16, kind="ExternalInput").ap()
    cos = nc.dram_tensor("cos", [128, ST, 64], f32, kind="ExternalInput").ap()
    sin = nc.dram_tensor("sin", [128, ST, 64], f32, kind="ExternalInput").ap()
    tri = nc.dram_tensor("tri", [128, 4, SG], f32, kind="ExternalInput").ap()
    onesc = nc.dram_tensor("onesc", [128, 1], f32r, kind="ExternalInput").ap()
    onesr = nc.dram_tensor("onesr", [1, 128], f32r, kind="ExternalInput").ap()
    ident = nc.dram_tensor("ident", [128, 128], f32r, kind="ExternalInput").ap()
    yt = nc.dram_tensor("yt", [512, S], f32, kind="ExternalOutput").ap()

    with tile.TileContext(nc) as tc:
        with (
            tc.tile_pool(name="const", bufs=1) as constp,
            tc.tile_pool(name="resid", bufs=1) as resid,
            tc.tile_pool(name="dram", bufs=1, space="DRAM") as dram,
            tc.tile_pool(name="dram_l", bufs=4, space="DRAM") as dram_l,
        ):
            cos_sb = constp.tile([128, ST, 64], f32)
            sin_sb = constp.tile([128, ST, 64], f32)
            tri_sb = constp.tile([128, 4, SG], f32)
            onesc_sb = constp.tile([128, 1], f32r)
            onesr_sb = constp.tile([1, 128], f32r)
            ident_sb = constp.tile([128, 128], f32r)
            nc.sync.dma_start(cos_sb[:], cos)
            nc.sync.dma_start(sin_sb[:], sin)
            nc.sync.dma_start(tri_sb[:], tri)
            nc.sync.dma_start(onesc_sb[:], onesc)
            nc.sync.dma_start(onesr_sb[:], onesr)
            nc.sync.dma_start(ident_sb[:], ident)

            # residents across phases
            kt_sb = resid.tile([128, S], f32r)           # roped K^T  [d, s]
            v_sb = resid.tile([128, ST, HD], f32r)       # natural V  [s-in-tile, st, d]

            qt_spill = dram.tile([QH * 128, S], f32r)    # roped Q^T per head
            cc_in = [dram.tile([128, S], bf16, name=f"cc_in{h}") for h in range(QH)]
            cc_out = [dram.tile([NCORES * 128, S], bf16, addr_space="Shared",
                                name=f"cc_out{h}") for h in range(QH)]

            # ---------------- Phase 1: QKV projections + RoPE + transposes
            with (
                tc.tile_pool(name="wpool", bufs=1) as wpool,
                tc.tile_pool(name="xpool", bufs=2) as xpool,
                tc.tile_pool(name="p1tmp", bufs=3) as p1tmp,
                tc.tile_pool(name="p1psum", bufs=2, space="PSUM") as p1psum,
            ):
                wq_sb = wpool.tile([128, DK, QH * HD], bf16)
                wkv_sb = wpool.tile([128, DK, 2 * HD], bf16)
                for kt in range(DK):
                    nc.sync.dma_start(wq_sb[:, kt, :], wq[:, kt, :])
                    nc.sync.dma_start(wkv_sb[:, kt, :], wkv[:, kt, :])

                def emit_transposes(qnat, knat, ssl):
                    # deferred one s-tile so PE never waits on the DVE rope
                    qts = p1tmp.tile([128, QH, 128], f32r, name="qts")
                    for h in range(QH):
                        qtr = p1psum.tile([128, 128], f32r, name="qtr")
                        nc.tensor.transpose(qtr[:], qnat[:, h, :], ident_sb[:])
                        nc.vector.tensor_copy(qts[:, h, :], qtr[:])
                    nc.sync.dma_start(
                        qt_spill.rearrange("(h p) s -> p h s", p=128)[:, :, ssl],
                        qts[:])
                    ktr = p1psum.tile([128, 128], f32r, name="ktr")
                    nc.tensor.transpose(ktr[:], knat[:], ident_sb[:])
                    nc.vector.tensor_copy(kt_sb[:, ssl], ktr[:])

                pending = None
                for st in range(ST):
                    ssl = slice(st * 128, (st + 1) * 128)
                    xt_st = xpool.tile([128, DK, 128], bf16, name="xt_st")
                    nc.sync.dma_start(xt_st[:], xt[st])

                    q_ps = p1psum.tile([128, QH * HD], f32, name="q_ps")
                    kv_ps = p1psum.tile([128, 2 * HD], f32, name="kv_ps")
                    for kt in range(DK):
                        nc.tensor.matmul(q_ps[:], lhsT=xt_st[:, kt], rhs=wq_sb[:, kt],
                                         start=(kt == 0), stop=(kt == DK - 1))
                    for kt in range(DK):
                        nc.tensor.matmul(kv_ps[:], lhsT=xt_st[:, kt], rhs=wkv_sb[:, kt],
                                         start=(kt == 0), stop=(kt == DK - 1))
                    if pending is not None:
                        emit_transposes(*pending)

                    # RoPE on q (4 heads batched) during PSUM eviction.
                    qp = q_ps[:].rearrange("p (h d) -> p h d", h=QH)
                    qa, qb = qp[:, :, 0:64], qp[:, :, 64:128]
                    cbc = cos_sb[:, st:st + 1, :].to_broadcast([128, QH, 64])
                    sbc = sin_sb[:, st:st + 1, :].to_broadcast([128, QH, 64])
                    t1 = p1tmp.tile([128, QH, 64], f32, name="t1")
                    t2 = p1tmp.tile([128, QH, 64], f32, name="t2")
                    qnat = p1tmp.tile([128, QH, HD], f32r, name="qnat")
                    na, nb = qnat[:, :, 0:64], qnat[:, :, 64:128]
                    nc.vector.tensor_tensor(t1[:], qa, sbc, mybir.AluOpType.mult)
                    nc.vector.tensor_tensor(t2[:], qb, sbc, mybir.AluOpType.mult)
                    nc.vector.tensor_tensor(na, qa, cbc, mybir.AluOpType.mult)
                    nc.vector.tensor_tensor(nb, qb, cbc, mybir.AluOpType.mult)
                    nc.vector.tensor_tensor(na, na, t2[:], mybir.AluOpType.subtract)
                    nc.vector.tensor_tensor(nb, nb, t1[:], mybir.AluOpType.add)

                    # RoPE on k
                    ka, kb = kv_ps[:, 0:64], kv_ps[:, 64:128]
                    cb1 = cos_sb[:, st, :]
                    sb1 = sin_sb[:, st, :]
                    kt1 = p1tmp.tile([128, 64], f32, name="kt1")
                    kt2 = p1tmp.tile([128, 64], f32, name="kt2")
                    knat = p1tmp.tile([128, HD], f32r, name="knat")
                    kna, knb = knat[:, 0:64], knat[:, 64:128]
                    nc.vector.tensor_tensor(kt1[:], ka, sb1, mybir.AluOpType.mult)
                    nc.vector.tensor_tensor(kt2[:], kb, sb1, mybir.AluOpType.mult)
                    nc.vector.tensor_tensor(kna, ka, cb1, mybir.AluOpType.mult)
                    nc.vector.tensor_tensor(knb, kb, cb1, mybir.AluOpType.mult)
                    nc.vector.tensor_tensor(kna, kna, kt2[:], mybir.AluOpType.subtract)
                    nc.vector.tensor_tensor(knb, knb, kt1[:], mybir.AluOpType.add)

                    # V natural, straight copy
                    nc.vector.tensor_copy(v_sb[:, st, :], kv_ps[:, HD:2 * HD])

                    pending = (qnat, knat, ssl)
                emit_transposes(*pending)

            # ---------------- Phase 2: attention per (head, group) + wo preload
            with tc.tile_pool(name="wopool", bufs=1) as wopool:
                wo_sb = wopool.tile([128, DK, 512], bf16)
                nc.sync.dma_start(wo_sb[:], wo)

                with (
                    tc.tile_pool(name="p2tmp", bufs=3) as p2tmp,
                    tc.tile_pool(name="p2lb", bufs=2) as p2lb,
                    tc.tile_pool(name="p2psum", bufs=2, space="PSUM") as p2psum,
                    tc.tile_pool(name="p2opsum", bufs=2, space="PSUM") as p2opsum,
                ):
                    for h in range(QH):
                        for g in range(G):
                            gsl = slice(g * SG, (g + 1) * SG)
                            qt_g = p2tmp.tile([128, SG], f32r, name="qt_g")
                            nc.sync.dma_start(
                                qt_g[:], qt_spill[h * 128:(h + 1) * 128, gsl])

                            ot_ps = p2opsum.tile([128, SG], f32, name="ot_ps", bufs=2)
                            l_ps = p2opsum.tile([1, SG], f32, name="l_ps")
                            nk = 4 * g + 4
                            DEPTH = 3
                            st_tiles = {}

                            def do_st(j, qt_g=qt_g, st_tiles=st_tiles):
                                stp = p2psum.tile([128, SG], f32, name="st_ps",
                                                  tag="st_ps", bufs=DEPTH)
                                nc.tensor.matmul(
                                    stp[:], lhsT=kt_sb[:, j * 128:(j + 1) * 128],
                                    rhs=qt_g[:], start=True, stop=True)
                                st_tiles[j] = stp

                            for j in range(min(DEPTH, nk)):
                                do_st(j)
                            for j in range(nk):
                                st_ps = st_tiles.pop(j)
                                put = p2tmp.tile([128, SG], f32r, name="put")
                                nc.scalar.activation(put[:], st_ps[:],
                                                     mybir.ActivationFunctionType.Exp,
                                                     scale=SCALE)
                                if j >= 4 * g:
                                    nc.vector.tensor_tensor(put[:], put[:],
                                                            tri_sb[:, j - 4 * g, :],
                                                            mybir.AluOpType.mult)
                                nc.tensor.matmul(ot_ps[:], lhsT=v_sb[:, j, :],
                                                 rhs=put[:],
                                                 start=(j == 0), stop=(j == nk - 1))
                                nc.tensor.matmul(l_ps[:], lhsT=onesc_sb[:],
                                                 rhs=put[:],
                                                 start=(j == 0), stop=(j == nk - 1))
                                if j + DEPTH < nk:
                                    do_st(j + DEPTH)

                            linv_f = p2lb.tile([1, SG], f32, name="linv_f")
                            nc.vector.reciprocal_approx_fast(linv_f[:], l_ps[:])
                            linv_r = p2lb.tile([1, SG], f32r, name="linv_r")
                            nc.vector.tensor_copy(linv_r[:], linv_f[:])
                            lb_ps = p2psum.tile([128, SG], f32, name="lb_ps",
                                                tag="lb_ps", bufs=1)
                            nc.tensor.matmul(lb_ps[:], lhsT=onesr_sb[:],
                                             rhs=linv_r[:], start=True, stop=True)
                            lb_sb = p2lb.tile([128, SG], f32, name="lb_sb")
                            nc.vector.tensor_copy(lb_sb[:], lb_ps[:])
                            on_sb = p2tmp.tile([128, SG], bf16, name="on_sb")
                            nc.vector.tensor_tensor(on_sb[:], ot_ps[:], lb_sb[:],
                                                    mybir.AluOpType.mult)
                            nc.sync.dma_start(cc_in[h][:, gsl], on_sb[:])

                        nc.gpsimd.collective_compute(
                            "AllGather", mybir.AluOpType.bypass,
                            ins=[cc_in[h].opt()], outs=[cc_out[h].opt()],
                            replica_groups=[list(range(NCORES))],
                        )

                # ---------------- Phase 3: yT = wo^T-contract @ O^T_full
                with (
                    tc.tile_pool(name="p3tmp", bufs=3) as p3tmp,
                    tc.tile_pool(name="p3out", bufs=2) as p3out,
                    tc.tile_pool(name="p3psum", bufs=2, space="PSUM") as p3psum,
                ):
                    for sq in range(4):
                        sqsl = slice(sq * 512, (sq + 1) * 512)
                        y_ps = p3psum.tile([128, 4, 512], f32, name="y_ps")
                        for h in range(QH):
                            ot_h = p3tmp.tile([128, NCORES, 512], bf16, name="ot_h")
                            nc.sync.dma_start(
                                ot_h[:],
                                cc_out[h].rearrange("(r p) s -> p r s",
                                                    p=128)[:, :, sqsl])
                            for r in range(NCORES):
                                kt2 = h * NCORES + r
                                for dt in range(4):
                                    nc.tensor.matmul(
                                        y_ps[:, dt, :],
                                        lhsT=wo_sb[:, kt2, dt * 128:(dt + 1) * 128],
                                        rhs=ot_h[:, r, :],
                                        start=(kt2 == 0), stop=(kt2 == DK - 1))
                        for dt in range(4):
                            y_sb = p3out.tile([128, 512], f32, name="y_sb")
                            nc.vector.tensor_copy(y_sb[:], y_ps[:, dt, :])
                            nc.sync.dma_start(yt[dt * 128:(dt + 1) * 128, sqsl],
                                              y_sb[:])
    nc.compile()
    return nc


def _host_prep(inputs):
    x = np.asarray(inputs["x"], dtype=np.float32)
    wq = np.asarray(inputs["wq"], dtype=np.float32)
    wk = np.asarray(inputs["wk"], dtype=np.float32)
    wv = np.asarray(inputs["wv"], dtype=np.float32)
    wo = np.asarray(inputs["wo"], dtype=np.float32)
    cos = np.asarray(inputs["freqs_cos"], dtype=np.float32)
    sin = np.asarray(inputs["freqs_sin"], dtype=np.float32)
    mask = np.asarray(inputs["mask"], dtype=np.float32)

    # xt[st, p, kt, s] = x[128*st + s, 128*kt + p]
    xts = np.ascontiguousarray(
        x.reshape(ST, 128, DK, 128).transpose(0, 3, 2, 1))

    # de-interleave RoPE pairs within each head: evens then odds
    perm = np.concatenate([np.arange(0, HD, 2), np.arange(1, HD, 2)])

    cos_t = np.ascontiguousarray(cos.reshape(ST, 128, 64).transpose(1, 0, 2))
    sin_t = np.ascontiguousarray(sin.reshape(ST, 128, 64).transpose(1, 0, 2))

    # causal tile masks from the actual mask input (g-independent for causal)
    trif = np.empty((4, 128, SG), dtype=np.float32)
    for r in range(4):
        trif[r] = (mask[0:SG, 128 * r:128 * (r + 1)].T == 0.0).astype(np.float32)
    tri_t = np.ascontiguousarray(trif.transpose(1, 0, 2))

    def ktile(w):  # [D, m] -> [128, DK, m]
        return np.ascontiguousarray(
            w.reshape(DK, 128, w.shape[1]).transpose(1, 0, 2))

    in_maps = []
    for c in range(NCORES):
        wq_c = wq[:, 512 * c:512 * (c + 1)].reshape(D, QH, HD)[:, :, perm]
        wq_c = wq_c.reshape(D, QH * HD)
        wk_c = wk[:, 128 * c:128 * (c + 1)][:, perm]
        wv_c = wv[:, 128 * c:128 * (c + 1)]
        wkv_c = np.concatenate([wk_c, wv_c], axis=1)
        # wo rows reordered to (head, rank, d) to match per-head AllGather
        wo_c = wo[:, 512 * c:512 * (c + 1)]
        wo_c = wo_c.reshape(NCORES, QH, 128, 512).transpose(1, 0, 2, 3)
        wo_c = wo_c.reshape(D, 512)
        in_maps.append({
            "xt": xts.astype(ml_dtypes.bfloat16),
            "wq": ktile(wq_c).astype(ml_dtypes.bfloat16),
            "wkv": ktile(wkv_c).astype(ml_dtypes.bfloat16),
            "wo": ktile(wo_c).astype(ml_dtypes.bfloat16),
            "cos": cos_t,
            "sin": sin_t,
            "tri": tri_t,
            "onesc": np.ones((128, 1), dtype=np.float32),
            "onesr": np.ones((1, 128), dtype=np.float32),
            "ident": np.eye(128, dtype=np.float32),
        })
    return in_maps


def _run(inputs, trace=False, tmpdir=None):
    from concourse import bass_utils
    if "nc" not in _CACHE:
        _CACHE["nc"] = _build()
    nc = _CACHE["nc"]
    in_maps = _host_prep(inputs)
    res = bass_utils.run_bass_kernel_spmd(
        nc, in_maps, core_ids=list(range(NCORES)), trace=trace, tmpdir=tmpdir)
    yts = [res.results[c]["yt"] for c in range(NCORES)]
    y = np.concatenate([t.T for t in yts], axis=1).astype(np.float32)
    return y.reshape(1, S, D), res


def kernel(**inputs):
    y, _ = _run(inputs, trace=False)
    return y
